# revision 1
# baseline (speedup 1.0000x reference)
"""Power attention (p=2) layer for Trainium2, 8 NeuronCores.

Math: the reference recurrence
    state_t = g*state_{t-1} + spow2(k_t) v_t^T ;  out_t = spow2(q_t) . state_t
with spow2 the symmetric-power feature map satisfies
    spow2(q).spow2(k) = (q.k)^2
so  out_t = sum_{s<=t} g^(t-s) (q_t.k_s)^2 v_s  -- masked quadratic attention.

Decay handling: scale q_t by g^(t/2) and k_s by g^(-s/2); then
    (kt_s . qt_t)^2 = g^(t-s) (q_t.k_s)^2
exactly, so no decay matrix is ever materialized; only an upper-triangular
0/1 mask on the 8 diagonal 128x128 blocks is needed.

Sharding: core c -> batch b=c//4, head group g=c%4 (4 heads = 128 qkv cols).
Each core computes a partial [S, HIDDEN] output (its heads' contribution via
its o_w row block); host sums the 4 partials per batch and adds o_b.
"""

import sys

import numpy as np

sys.path.insert(0, "/opt/trn_rl_repo")

import concourse.bass as bass  # noqa: E402
import concourse.tile as tile  # noqa: E402
from concourse import bacc  # noqa: E402
from concourse import mybir  # noqa: E402
from concourse import bass_utils  # noqa: E402
from concourse.bass import ts  # noqa: E402

B, S, HIDDEN = 2, 1024, 512
NH, HD = 16, 32
GAMMA = 0.9
NCORES = 8
HPC = 4            # heads per core
CW = HPC * HD      # 128 qkv columns per core
NKT = HIDDEN // 128  # 4 contraction tiles over hidden
NST = S // 128       # 8 seq tiles of 128
STRIP = 512          # t-strip width (one PSUM bank of f32)
NSTRIP = S // STRIP  # 2

F32 = mybir.dt.float32
F32R = mybir.dt.float32r
F16 = mybir.dt.float16
AF = mybir.ActivationFunctionType
OP = mybir.AluOpType


def _r(ap):
    """Retag an f32 AP as float32r (full-rate PE streaming, tf32-ish precision)."""
    return ap.bitcast(F32R)


def _bcast_dram(ap, p=128):
    """DRAM-side AP replicating a [1, N] tensor across p partitions (DMA only)."""
    return bass.AP(tensor=ap.tensor, offset=ap.offset, ap=[[0, p]] + list(ap.ap[1:]))


def _free_bcast(ap2d, times):
    """[P, N] AP -> [P, times, N] AP broadcasting along a new middle free dim."""
    part, free = ap2d.ap[0], list(ap2d.ap[1:])
    return bass.AP(tensor=ap2d.tensor, offset=ap2d.offset,
                   ap=[part, [0, times]] + free)


def _build_program():
    nc = bacc.Bacc("TRN2", debug=False, target_bir_lowering=False)

    xT = nc.dram_tensor("xT", [HIDDEN, S], F32R, kind="ExternalInput").ap()
    # wqkv: [hid, qw|kw|vw] packed; qkvb: [qb|kb|vb] columns; m16: [dtri|ident]
    wqkv = nc.dram_tensor("wqkv", [HIDDEN, 3 * CW], F32R, kind="ExternalInput").ap()
    qkvb = nc.dram_tensor("qkvb", [CW, 3], F32, kind="ExternalInput").ap()
    ow = nc.dram_tensor("ow", [CW, HIDDEN], F32R, kind="ExternalInput").ap()
    gqgk = nc.dram_tensor("gqgk", [128, 2, S], F32, kind="ExternalInput").ap()
    m16 = nc.dram_tensor("m16", [128, 256], F16, kind="ExternalInput").ap()
    yp = nc.dram_tensor("yp", [S, HIDDEN], F16, kind="ExternalOutput").ap()

    AST = 256     # attention window per s-tile (diag tile + 1 band tile)
    OST = 256     # output accumulation strip

    with tile.TileContext(nc) as tc:
        with (
            tc.tile_pool(name="const", bufs=1) as const,
            tc.tile_pool(name="apool", bufs=6) as apool,
            tc.tile_pool(name="ypool", bufs=3) as ypool,
            tc.tile_pool(name="qkvp", bufs=2, space="PSUM") as qkvp,
            tc.tile_pool(name="ps4p", bufs=2, space="PSUM") as ps4p,
            tc.tile_pool(name="accp", bufs=2, space="PSUM") as accp,
        ):
            # ---- loads: few big DMAs (each DMA costs ~0.6us of HWDGE
            # issue time), ordered by first use ----
            qkvb_sb = const.tile([CW, 3], F32)
            nc.sync.dma_start(qkvb_sb, qkvb)
            m16_sb = const.tile([128, 256], F16)
            nc.scalar.dma_start(m16_sb, m16)
            qb_sb, kb_sb, vb_sb = (qkvb_sb[:, i : i + 1] for i in range(3))
            dtri_sb, ident_sb = m16_sb[:, 0:128], m16_sb[:, 128:256]

            wqkv_sb = const.tile([128, NKT, 3 * CW], F32R)
            wr = wqkv.rearrange("(k p) e -> p k e", p=128)
            xT_sb = const.tile([128, NKT, S], F32R)
            xTr = xT.rearrange("(k p) n -> p k n", p=128)
            gqgk_sb = const.tile([128, 2, S], F32)
            gq_sb, gk_sb = gqgk_sb[:, 0, :], gqgk_sb[:, 1, :]
            nc.sync.dma_start(gqgk_sb[:, :, ts(0, STRIP)], gqgk[:, :, ts(0, STRIP)])
            nc.scalar.dma_start(wqkv_sb[:, 0:2, :], wr[:, 0:2, :])
            nc.sync.dma_start(xT_sb[:, 0, ts(0, STRIP)], xTr[:, 0, ts(0, STRIP)])
            nc.scalar.dma_start(xT_sb[:, 1, ts(0, STRIP)], xTr[:, 1, ts(0, STRIP)])
            nc.sync.dma_start(wqkv_sb[:, 2:4, :], wr[:, 2:4, :])
            nc.scalar.dma_start(xT_sb[:, 2, ts(0, STRIP)], xTr[:, 2, ts(0, STRIP)])
            nc.sync.dma_start(xT_sb[:, 3, ts(0, STRIP)], xTr[:, 3, ts(0, STRIP)])
            nc.scalar.dma_start(gqgk_sb[:, :, ts(1, STRIP)], gqgk[:, :, ts(1, STRIP)])
            nc.sync.dma_start(xT_sb[:, :, ts(1, STRIP)], xTr[:, :, ts(1, STRIP)])
            ow_sb = const.tile([CW, HIDDEN], F32R)
            nc.scalar.dma_start(ow_sb, ow)
            qw_sb = wqkv_sb[:, :, 0:CW]
            kw_sb = wqkv_sb[:, :, CW : 2 * CW]
            vw_sb = wqkv_sb[:, :, 2 * CW : 3 * CW]

            qT_sb = const.tile([CW, S], F32R, tag="qT")
            kT_sb = const.tile([CW, S], F32R, tag="kT")
            vT_sb = const.tile([CW, S], F16, tag="vT")
            v_sb = const.tile([128, NST, CW], F16, tag="v")
            outT_sb = const.tile([CW, S], F32R, tag="outT")

            def qk_strip(T):
                tsl = ts(T, STRIP)
                for w_sb, b_sb, g_sb, dst in (
                    (qw_sb, qb_sb, gq_sb, qT_sb),
                    (kw_sb, kb_sb, gk_sb, kT_sb),
                ):
                    ps = qkvp.tile([128, STRIP], F32, tag="mm")
                    for k in range(NKT):
                        nc.tensor.matmul(
                            ps,
                            _r(w_sb[:, k, :]),
                            _r(xT_sb[:, k, tsl]),
                            start=(k == 0),
                            stop=(k == NKT - 1),
                        )
                    # dst = (ps + bias[e]) * g[t]
                    nc.vector.scalar_tensor_tensor(
                        out=dst[:, tsl], in0=ps, scalar=b_sb[:, 0:1],
                        in1=g_sb[:, tsl], op0=OP.add, op1=OP.mult,
                    )

            def vt_strip(T):
                # V^T [d, t] strip, then PE-transpose each 128-block to V [s, d]
                tsl = ts(T, STRIP)
                ps = qkvp.tile([128, STRIP], F32, tag="mm")
                for k in range(NKT):
                    nc.tensor.matmul(
                        ps,
                        _r(vw_sb[:, k, :]),
                        _r(xT_sb[:, k, tsl]),
                        start=(k == 0),
                        stop=(k == NKT - 1),
                    )
                nc.scalar.activation(vT_sb[:, tsl], ps, AF.Identity, bias=vb_sb[:, 0:1])
                for a in range(4 * T, 4 * T + 4):
                    tp = qkvp.tile([128, 128], F16, tag="mm")
                    nc.tensor.transpose(tp, vT_sb[:, ts(a, 128)], ident_sb)
                    if a % 2 == 0:
                        nc.vector.tensor_copy(v_sb[:, a, :], tp)
                    else:
                        nc.scalar.activation(v_sb[:, a, :], tp, AF.Copy)

            oTs = {}

            def att_s(a):
                # P~ window for s-tile a: t in [128a, 128a+256); diag tile at
                # cols [0,128), band tile at [128,256). gamma decay makes
                # older tiles negligible.
                w0 = 128 * a
                w1 = min(w0 + AST, S)
                n = w1 - w0
                # two head-pair PSUM groups; concurrent row-group matmuls
                # must land in distinct PSUM banks (same-bank pairs hang)
                a4 = apool.tile([128, HPC, n], F16, tag="a")
                for g, ps2 in ((0, ps4p.tile([128, 2, 512], F32, tag="ps4", name=f"psA{a}")),
                               (1, ps4p.tile([128, 2, 512], F32, tag="ps4", name=f"psB{a}"))):
                    for hh in range(2):
                        h = 2 * g + hh
                        nc.tensor.matmul(
                            ps2[:, hh, 0:n],
                            _r(kT_sb[ts(h, 32), ts(a, 128)]),
                            _r(qT_sb[ts(h, 32), w0:w1]),
                            start=True,
                            stop=True,
                            tile_position=(32 * h, 0),
                        )
                    # upper-tri mask on the diag block, in PSUM (before the
                    # square: pre-mask garbage would overflow f16)
                    nc.vector.scalar_tensor_tensor(
                        out=ps2[:, :, 0:128], in0=ps2[:, :, 0:128],
                        scalar=1.0, in1=_free_bcast(dtri_sb, 2),
                        op0=OP.mult, op1=OP.mult,
                    )
                    nc.scalar.square(a4[:, 2 * g : 2 * g + 2, :], ps2[:, :, 0:n])
                # accumulate into overlapping output strips
                for T in sorted({w0 // OST, (w1 - 1) // OST}):
                    if T not in oTs:
                        oTs[T] = accp.tile([128, OST], F32, tag="acc", name=f"oT{T}")
                        nc.vector.memset(oTs[T], 0.0)
                    lo, hi = max(w0, OST * T), min(w1, OST * (T + 1))
                    for h in range(HPC):
                        # pure accumulate onto the zeroed bank: correct for
                        # both has_written states, and every write region is
                        # uniform (no partial pending-zero)
                        nc.tensor.matmul(
                            oTs[T][ts(h, 32), lo - OST * T : hi - OST * T],
                            v_sb[:, a, ts(h, 32)],
                            a4[:, h, lo - w0 : hi - w0],
                            start=False,
                            stop=(a == min(2 * T + 1, NST - 1)),
                            tile_position=(0, 32 * h),
                            skip_group_check=True,
                        )

            def close_strip(T):
                base = OST * T
                oT = oTs.pop(T)
                if T % 2 == 0:
                    nc.vector.tensor_copy(outT_sb[:, base : base + OST], oT)
                else:
                    nc.scalar.activation(outT_sb[:, base : base + OST], oT, AF.Copy)
                for j2 in range(2 * T, 2 * T + 2):
                    ps = qkvp.tile([128, HIDDEN], F32, tag="mm")
                    nc.tensor.matmul(ps, _r(outT_sb[:, ts(j2, 128)]), _r(ow_sb),
                                     start=True, stop=True)
                    y_sb = ypool.tile([128, HIDDEN], F16, tag="y")
                    if j2 % 2 == 0:
                        nc.scalar.activation(y_sb, ps, AF.Copy)
                    else:
                        nc.vector.tensor_copy(y_sb, ps)
                    nc.sync.dma_start(yp[ts(j2, 128), :], y_sb)

            # ---- interleaved schedule ----
            # strip T (256 cols) closes after its last contributor a=2T+1
            qk_strip(0)
            vt_strip(0)
            att_s(0)
            qk_strip(1)
            att_s(1)
            close_strip(0)
            att_s(2)
            vt_strip(1)
            att_s(3)
            close_strip(1)
            att_s(4); att_s(5)
            close_strip(2)
            att_s(6); att_s(7)
            close_strip(3)

    nc.compile()
    return nc


_CACHED = None


def _get_program():
    global _CACHED
    if _CACHED is None:
        _CACHED = _build_program()
    return _CACHED


def _in_maps(x, q_w, q_b, k_w, k_b, v_w, v_b, o_w, o_b):
    x = np.ascontiguousarray(np.asarray(x, np.float32))
    t = np.arange(S, dtype=np.float64)
    gq_v = (GAMMA ** (t / 2)).astype(np.float32)
    gk_v = (GAMMA ** (-t / 2)).astype(np.float32)
    gqgk_v = np.ascontiguousarray(
        np.broadcast_to(np.stack([gq_v, gk_v]), (128, 2, S))
    )
    m16_v = np.concatenate(
        [np.triu(np.ones((128, 128), np.float16)), np.eye(128, dtype=np.float16)],
        axis=1,
    )
    qw_f, kw_f, vw_f = (np.asarray(w, np.float32) for w in (q_w, k_w, v_w))
    qb_f, kb_f, vb_f = (np.asarray(b, np.float32) for b in (q_b, k_b, v_b))
    ow_f = np.asarray(o_w, np.float32)

    in_maps = []
    for c in range(NCORES):
        b, g = divmod(c, HPC)
        cs = slice(g * CW, (g + 1) * CW)
        in_maps.append(
            {
                "xT": np.ascontiguousarray(x[b].T),
                "wqkv": np.ascontiguousarray(
                    np.concatenate([qw_f[:, cs], kw_f[:, cs], vw_f[:, cs]], axis=1)
                ),
                "qkvb": np.ascontiguousarray(
                    np.stack([qb_f[cs], kb_f[cs], vb_f[cs]], axis=1)
                ),
                "ow": np.ascontiguousarray(ow_f[cs, :]),
                "gqgk": gqgk_v,
                "m16": m16_v,
            }
        )
    return in_maps


def _gather(res, o_b):
    parts = [res.results[c]["yp"] for c in range(NCORES)]
    out = np.empty((B, S, HIDDEN), np.float32)
    ob = np.asarray(o_b, np.float32)
    for b in range(B):
        out[b] = (
            parts[4 * b].astype(np.float32)
            + parts[4 * b + 1].astype(np.float32)
            + parts[4 * b + 2].astype(np.float32)
            + parts[4 * b + 3].astype(np.float32)
            + ob
        )
    return out


def kernel(x, q_w, q_b, k_w, k_b, v_w, v_b, o_w, o_b):
    in_maps = _in_maps(x, q_w, q_b, k_w, k_b, v_w, v_b, o_w, o_b)
    nc = _get_program()
    res = bass_utils.run_bass_kernel_spmd(nc, in_maps, core_ids=list(range(NCORES)))
    return _gather(res, o_b)


def run_traced(x, q_w, q_b, k_w, k_b, v_w, v_b, o_w, o_b):
    """Like kernel() but tries to collect an NTFF profile (may be absent)."""
    in_maps = _in_maps(x, q_w, q_b, k_w, k_b, v_w, v_b, o_w, o_b)
    nc = _get_program()
    try:
        res = bass_utils.run_bass_kernel_spmd(
            nc, in_maps, core_ids=list(range(NCORES)), trace=True
        )
    except Exception:
        res = bass_utils.run_bass_kernel_spmd(
            nc, in_maps, core_ids=list(range(NCORES))
        )
    res.gathered = _gather(res, o_b)
    return res


def cost_model_time_ns():
    """Per-core makespan from the instruction cost model (no NTFF on axon)."""
    from concourse.timeline_sim import TimelineSim

    return TimelineSim(_get_program(), trace=False).simulate()


if __name__ == "__main__":
    rng = np.random.default_rng(0)
    ins = {
        "x": rng.standard_normal((B, S, HIDDEN), dtype=np.float32),
        "q_w": rng.standard_normal((HIDDEN, HIDDEN), dtype=np.float32) * 0.04,
        "q_b": rng.standard_normal(HIDDEN, dtype=np.float32) * 0.04,
        "k_w": rng.standard_normal((HIDDEN, HIDDEN), dtype=np.float32) * 0.04,
        "k_b": rng.standard_normal(HIDDEN, dtype=np.float32) * 0.04,
        "v_w": rng.standard_normal((HIDDEN, HIDDEN), dtype=np.float32) * 0.04,
        "v_b": rng.standard_normal(HIDDEN, dtype=np.float32) * 0.04,
        "o_w": rng.standard_normal((HIDDEN, HIDDEN), dtype=np.float32) * 0.04,
        "o_b": rng.standard_normal(HIDDEN, dtype=np.float32) * 0.04,
    }
    out = kernel(**ins)
    print("kernel ran, out shape", out.shape, "norm", np.linalg.norm(out))

